# revision 23
# baseline (speedup 1.0000x reference)
"""MCR2 variational loss on 8 Trainium2 NeuronCores.

Math (reference):
  loss_R   = 0.5 * logdet(I + d/(n*eps) * Z.T @ Z)
  loss_Rc  = 0.5 * sum_k(trPi_k * sum_p log1p(d/(trPi_k*eps) * relu(A)_kp)) / n
  loss_reg = 0.5 * sum_k || G_k - Un diag(relu(A)_k) Un.T ||_F^2,
             G_k = Z.T diag(Pi[:,k]) Z
  out = (-(loss_R - loss_Rc - mu*loss_reg), loss_R, loss_Rc, loss_reg)

Fast path (Pi one-hot, the setup_inputs case): rows partition into 10
classes, so each masked Gram is a plain Gram over that class's rows and
the full Gram (for loss_R) is the sum of the ten. The host sorts rows by
class, deals each class's rows evenly across the 8 cores, and zero-pads
each (class, core) block to whole 128-row chunks (zero rows contribute
nothing). Per core the device then streams ~100 bf16 chunks ([128, 128]
each, partition-major in DRAM so DMA reads are contiguous) and runs ONE
N=128 matmul per chunk, accumulating class k's chunks into a per-class
PSUM bank. As each class finishes, its [128,128] Gram is copied to SBUF
(cast to bf16) and DMA'd out on the scalar HWDGE ring while later
classes still compute. Host sums the 8 partial G stacks and runs the
tiny O(k*d^2) epilogue (slogdet, compress, reg) in float64.

General path (Pi not one-hot): weighted-copy kernel — DVE builds
Pi_k * Z per chunk, wide matmuls accumulate the 10 masked Grams and the
full Gram.
"""

import math
import sys

if "/opt/trn_rl_repo" not in sys.path:
    sys.path.insert(0, "/opt/trn_rl_repo")

import ml_dtypes
import numpy as np

import concourse.bacc as bacc
import concourse.mybir as mybir
import concourse.tile as tile
from concourse import bass_utils

# Problem constants (hardcoded per harness contract).
N, D, K = 100000, 128, 10
EPS, MU = 0.5, 1.0
N_CORES = 8

_NC_CACHE = {}

# Device input dtype for the sorted fast path. fp8e4m3 halves HBM input
# traffic; host-simulated quantization error on the final losses is
# ~1.5e-3, far inside the 2e-2 gate (bf16: ~1e-6).
IN_DT = "fp8"  # "fp8" | "bf16"


# ---------------------------------------------------------------------------
# Fast path: Pi is one-hot -> sort rows by class, plain per-class Grams.
# ---------------------------------------------------------------------------

N_WARM = 9  # PE warmup matmuls (HAM clock-gate) while the first DMA lands


def _group_sizes(c_tot):
    """Input DMA slab sizes: small first slabs so compute starts early,
    16-chunk middle slabs, small final slab so the last chunks'
    completion semaphore doesn't gate the pipeline drain. Slabs
    alternate between the two HWDGE rings in consumption order, so
    neither ring runs far ahead of what the PE needs next."""
    head = [2, 2, 6, 8, 10, 12]
    tail = [24, 20]
    if c_tot <= sum(head) + sum(tail):
        sizes = []
        rem = c_tot
        while rem > 0:
            s = min(8, rem)
            sizes.append(s)
            rem -= s
        return sizes
    rem = c_tot - sum(head) - sum(tail)
    mid = []
    while rem > 0:
        s = min(16, rem)
        mid.append(s)
        rem -= s
    return head + mid + tail


def _build_nc_sorted(layout):
    """layout: tuple of 10 ints — 128-row chunks per class per core."""
    f32 = mybir.dt.float32
    bf16 = mybir.dt.bfloat16

    c_tot = sum(layout)
    starts = [0] * K
    for k in range(1, K):
        starts[k] = starts[k - 1] + layout[k - 1]
    class_of = []
    for k in range(K):
        class_of += [k] * layout[k]

    in_dt = mybir.dt.float8e4 if IN_DT == "fp8" else bf16

    nc = bacc.Bacc("TRN2", target_bir_lowering=False, debug=False)
    # Partition-major: row p holds chunk-row p of every chunk, so a slab
    # DMA reads a contiguous [128, g*128] region per partition.
    Zin = nc.dram_tensor("Zin", [128, c_tot * D], in_dt, kind="ExternalInput")
    # Separate output tensors per PSUM bank so the output DMAs have no
    # WAW dependency on each other and stream in parallel.
    G0 = nc.dram_tensor("G0", [128, 4 * D], bf16, kind="ExternalOutput")
    G1 = nc.dram_tensor("G1", [128, 4 * D], bf16, kind="ExternalOutput")
    G2 = nc.dram_tensor("G2", [128, D], bf16, kind="ExternalOutput")
    G3 = nc.dram_tensor("G3", [128, D], bf16, kind="ExternalOutput")

    with tile.TileContext(nc) as tc:
        with (
            tc.tile_pool(name="zb", bufs=1) as zpool,
            tc.tile_pool(name="res", bufs=1) as opool,
            tc.tile_pool(name="warm", bufs=1) as warmpool,
            tc.tile_pool(name="ps", bufs=5, space="PSUM") as pspool,
        ):
            # Whole shard lives in SBUF (25KB/partition); slab DMAs write
            # disjoint column ranges so matmuls on early slabs overlap
            # later slabs' transfers.
            zin = zpool.tile([128, c_tot * D], in_dt, name="zin")
            out = opool.tile([128, K * D], bf16, name="out")

            # PE warmup: the HAM clock gate only lifts to 2.4 GHz after
            # ~3.4µs of gap-free PE activity, and only re-throttles after
            # ~3.4µs of continuous idle. So: a dense warmup burst while
            # the first DMAs land, then short gap-filler matmuls between
            # the first slabs' chunks to bridge early data hiccups until
            # the window fires. After that the whole kernel runs warm.
            wsrc = warmpool.tile([128, 256], bf16, name="wsrc")
            wps = pspool.tile([128, 256], f32, name="wps", tag="ps")
            nc.gpsimd.memset(wsrc[:], 0.0)
            for _ in range(N_WARM):
                nc.tensor.matmul(wps[:], wsrc[:, 0:128], wsrc[:], start=True,
                                 stop=True, skip_group_check=True)

            def filler(n):
                for _ in range(n):
                    nc.tensor.matmul(wps[:, 0:128], wsrc[:, 0:128],
                                     wsrc[:, 0:128], start=True, stop=True,
                                     skip_group_check=True)

            # Input slabs alternate between the two HWDGE rings (SP via
            # nc.sync, ACT via nc.scalar) so descriptor generation and
            # packet streams run on both in parallel.
            g0 = 0
            for gi, sz in enumerate(_group_sizes(c_tot)):
                eng = nc.sync if gi % 2 == 0 else nc.scalar
                eng.dma_start(zin[:, g0 * D:(g0 + sz) * D],
                              Zin[:, g0 * D:(g0 + sz) * D])
                g0 += sz

            # Classes pack 4-per-PSUM-bank (classes run sequentially, so
            # a later class's group-start bank-clear only touches
            # finished classes' has_written bits, never live data). One
            # cast + one output DMA per bank: no PSUM slot cycling (no
            # PE drains), and the PE is always writing a different bank
            # than the one the DVE reads. Early banks' outputs are
            # hidden under compute on the sync ring; the last bank rides
            # the scalar ring on the tail.
            psA = pspool.tile([128, 4 * D], f32, name="psA", tag="ps")
            psB = pspool.tile([128, 4 * D], f32, name="psB", tag="ps")
            psC = pspool.tile([128, D], f32, name="psC", tag="ps")
            psD = pspool.tile([128, D], f32, name="psD", tag="ps")
            banks = {k: (psA, k) for k in range(4)}
            banks.update({k: (psB, k - 4) for k in range(4, 8)})
            banks[8] = (psC, 0)
            banks[9] = (psD, 0)
            bank_last = {3: (psA, 4, G0, nc.sync),
                         7: (psB, 4, G1, nc.sync),
                         8: (psC, 1, G2, nc.scalar),
                         9: (psD, 1, G3, nc.scalar)}

            # Gap fillers after the first slabs' chunks (slab boundaries
            # are where early data hiccups stall the PE).
            slab_fill = {0: 2, 1: 2, 2: 2, 3: 1, 4: 1, 5: 1}
            slab_bounds = []
            g0 = 0
            for sz in _group_sizes(c_tot):
                slab_bounds.append(g0 + sz)
                g0 += sz

            for idx in range(c_tot):
                k = class_of[idx]
                j = idx - starts[k]
                pst, off = banks[k]
                zc = zin[:, idx * D:(idx + 1) * D]
                nc.tensor.matmul(pst[:, off * D:(off + 1) * D], zc, zc,
                                 start=(j == 0), stop=(j == layout[k] - 1),
                                 skip_group_check=True)
                if j == layout[k] - 1 and k in bank_last:
                    pst, width, Gt, eng = bank_last[k]
                    osl = out[:, (k + 1 - width) * D:(k + 1) * D]
                    nc.vector.tensor_copy(osl, pst[:, 0:width * D])
                    eng.dma_start(Gt[:], osl)
                if idx + 1 in slab_bounds:
                    si = slab_bounds.index(idx + 1)
                    if si in slab_fill:
                        filler(slab_fill[si])

    nc.compile()
    return nc


def _layout_from_counts(counts):
    return tuple(
        max(1, math.ceil(math.ceil(int(c) / N_CORES) / 128)) for c in counts
    )


def _make_in_maps_sorted(Z, labels, counts, layout):
    c_tot = sum(layout)
    chunk_off = np.concatenate([[0], np.cumsum(layout)]).astype(np.int64)
    order = np.argsort(labels, kind="stable")
    np_dt = ml_dtypes.float8_e4m3 if IN_DT == "fp8" else ml_dtypes.bfloat16
    Zb = Z.astype(np_dt)

    per_core = np.zeros((N_CORES, c_tot * 128, D), np_dt)
    cum = 0
    for k in range(K):
        rows_k = order[cum:cum + counts[k]]
        cum += counts[k]
        q, r = divmod(int(counts[k]), N_CORES)
        s = 0
        base = int(chunk_off[k]) * 128
        for c in range(N_CORES):
            n_c = q + (1 if c < r else 0)
            if n_c:
                per_core[c, base:base + n_c] = Zb[rows_k[s:s + n_c]]
            s += n_c
    Zin = np.ascontiguousarray(
        per_core.reshape(N_CORES, c_tot, 128, D)
        .transpose(0, 2, 1, 3)
        .reshape(N_CORES, 128, c_tot * D)
    )
    return [{"Zin": Zin[c]} for c in range(N_CORES)]


# ---------------------------------------------------------------------------
# General path (Pi not one-hot): weighted-copy kernel from the baseline.
# ---------------------------------------------------------------------------

CHUNKS = 98                    # 128-row chunks per core
SHARD = CHUNKS * 128           # 12544 rows per core
NPAD = SHARD * N_CORES         # 100352 (zero-padded; zero rows contribute 0)
GROUP = 7                      # chunks per staged DMA group
NCLS = K + 1                   # 10 masked Grams + 1 full Gram


def _build_nc_general():
    f32 = mybir.dt.float32
    bf16 = mybir.dt.bfloat16

    nc = bacc.Bacc("TRN2", target_bir_lowering=False, debug=False)
    # Per-row payload: [Pi7*Z | Pi8*Z | Pi9*Z | Z_bf16] — classes 7..9
    # weighted on host. One DMA per group feeds everything; a single
    # N=512 matmul over the whole row computes G7, G8, G9 and the Gram.
    ZW = nc.dram_tensor("ZW", [SHARD, 4 * D], bf16, kind="ExternalInput")
    KD = K - 3  # classes 0..6 weighted on DVE; 7..9 host-weighted
    Pb = nc.dram_tensor("Pb", [128, CHUNKS, KD, 2], bf16, kind="ExternalInput")
    G = nc.dram_tensor("G", [D, NCLS * D], f32, kind="ExternalOutput")

    with tile.TileContext(nc) as tc:
        with (
            tc.tile_pool(name="zbf", bufs=6) as zbpool,
            tc.tile_pool(name="wgt", bufs=4) as wpool,
            tc.tile_pool(name="pi", bufs=1) as pipool,
            tc.tile_pool(name="res", bufs=1) as opool,
            tc.tile_pool(name="warm", bufs=1) as warmpool,
            tc.tile_pool(name="ps", bufs=1, space="PSUM") as pspool,
        ):
            psA = pspool.tile([128, 512], f32, name="psA")
            psB = pspool.tile([128, 384], f32, name="psB")
            psC = pspool.tile([128, 512], f32, name="psC")

            wsrc = warmpool.tile([128, 256], bf16, name="wsrc")
            wps = pspool.tile([128, 256], f32, name="wps")
            nc.gpsimd.memset(wsrc[:], 0.0)
            for _ in range(22):
                nc.tensor.matmul(wps[:], wsrc[:, 0:128], wsrc[:], start=True,
                                 stop=True, skip_group_check=True)

            Zr = ZW.rearrange("(c p) d -> p c d", p=128)

            pib = pipool.tile([128, CHUNKS, KD, 2], bf16, name="pib")
            nc.gpsimd.dma_start(pib[:, 0:1], Pb[:, 0:1])
            nc.gpsimd.dma_start(pib[:, 1:8], Pb[:, 1:8])
            nc.gpsimd.dma_start(pib[:, 8:29], Pb[:, 8:29])
            nc.gpsimd.dma_start(pib[:, 29:CHUNKS], Pb[:, 29:CHUNKS])

            sizes = [1] + [GROUP] * 12 + [5, 4, 3, 1]
            assert sum(sizes) == CHUNKS

            start_c = 0
            for gi, sz in enumerate(sizes):
                s0 = start_c
                start_c += sz
                zw = zbpool.tile([128, sz, 4 * D], bf16, name="zw", tag="zw")
                nc.sync.dma_start(zw[:], Zr[:, s0:s0 + sz, :])
                zb = zw[:, :, 3 * D:4 * D]

                wg = wpool.tile([128, sz, KD * D], bf16, name="wg", tag="wg")
                z_bc = zb.unsqueeze(2).broadcast_to([128, sz, KD, D])
                pi_bc = (
                    pib[:, s0:s0 + sz, :, :]
                    .unsqueeze(3)
                    .broadcast_to([128, sz, KD, 64, 2])
                )
                w5 = wg[:, :, 0:KD * D].rearrange(
                    "p c (k r t) -> p c k r t", k=KD, t=2
                )
                z5 = z_bc.rearrange("p c k (r t) -> p c k r t", t=2)
                nc.vector.tensor_mul(w5, z5, pi_bc)

                for c in range(sz):
                    idx = s0 + c
                    first = idx == 0
                    last = idx == CHUNKS - 1
                    zc = zw[:, c, 3 * D:4 * D]
                    w = wg[:, c, :]
                    nc.tensor.matmul(psA[:], zc, w[:, 0:512], start=first, stop=last)
                    nc.tensor.matmul(psB[:], zc, w[:, 512:896], start=first, stop=last)
                    nc.tensor.matmul(psC[:], zc, zw[:, c, :], start=first, stop=last)

                if gi <= 3:
                    for _ in range((10, 4, 3, 3)[gi]):
                        nc.tensor.matmul(wps[:], wsrc[:, 0:128], wsrc[:],
                                         start=True, stop=True,
                                         skip_group_check=True)

            out = opool.tile([128, NCLS * D], f32, name="out")
            nc.vector.tensor_copy(out[:, 0:512], psA[:])
            nc.scalar.copy(out[:, 512:896], psB[:])
            nc.sync.dma_start(G[:, 0:896], out[:, 0:896])
            nc.vector.tensor_copy(out[:, 896:1408], psC[:])
            nc.sync.dma_start(G[:, 896:1408], out[:, 896:1408])

    nc.compile()
    return nc


def _make_in_maps_general(Z, Pi):
    ZWpad = np.zeros((NPAD, 4 * D), ml_dtypes.bfloat16)
    for j in range(3):
        ZWpad[:N, j * D:(j + 1) * D] = (
            Pi[:, K - 3 + j:K - 2 + j] * Z
        ).astype(ml_dtypes.bfloat16)
    ZWpad[:N, 3 * D:4 * D] = Z.astype(ml_dtypes.bfloat16)
    Pipad = np.zeros((NPAD, K), np.float32)
    Pipad[:N] = Pi
    in_maps = []
    for i in range(N_CORES):
        zw = np.ascontiguousarray(ZWpad[i * SHARD:(i + 1) * SHARD])
        pt = (
            Pipad[i * SHARD:(i + 1) * SHARD, 0:K - 3]
            .reshape(CHUNKS, 128, K - 3)
            .transpose(1, 0, 2)
            .astype(ml_dtypes.bfloat16)
        )
        pb = np.ascontiguousarray(np.repeat(pt[..., None], 2, axis=-1))
        in_maps.append({"ZW": zw, "Pb": pb})
    return in_maps


# ---------------------------------------------------------------------------
# Shared driver / epilogue
# ---------------------------------------------------------------------------


def _get_nc(key):
    if key not in _NC_CACHE:
        if key == "general":
            _NC_CACHE[key] = _build_nc_general()
        else:
            _NC_CACHE[key] = _build_nc_sorted(key[1])
    return _NC_CACHE[key]


def _is_one_hot(Pi):
    if not np.all((Pi == 0.0) | (Pi == 1.0)):
        return False
    return bool(np.all(Pi.sum(axis=1) == 1.0))


def _plan(Z, Pi):
    """Returns (key, in_maps) for the right device path."""
    if _is_one_hot(Pi):
        labels = np.argmax(Pi, axis=1)
        counts = np.bincount(labels, minlength=K)
        layout = _layout_from_counts(counts)
        key = ("sorted", layout)
        in_maps = _make_in_maps_sorted(Z, labels, counts, layout)
    else:
        key = "general"
        in_maps = _make_in_maps_general(Z, Pi)
    return key, in_maps


def _run_device(key, in_maps, trace=False, tmpdir=None):
    nc = _get_nc(key)
    return bass_utils.run_bass_kernel_spmd(
        nc, in_maps, core_ids=list(range(N_CORES)), trace=trace, tmpdir=tmpdir
    )


def _gather(key, res):
    """Sum per-core partial Grams -> (Gk [K,D,D], Gram [D,D]) in float64."""
    if key == "general":
        G_all = np.zeros((D, NCLS * D), np.float64)
        for i in range(N_CORES):
            G_all += res.results[i]["G"].astype(np.float64)
        Gk = np.stack([G_all[:, k * D:(k + 1) * D] for k in range(K)])
        Gram = G_all[:, K * D:(K + 1) * D]
    else:
        G_all = np.zeros((D, K * D), np.float64)
        for i in range(N_CORES):
            G_all[:, 0:4 * D] += res.results[i]["G0"].astype(np.float64)
            G_all[:, 4 * D:8 * D] += res.results[i]["G1"].astype(np.float64)
            G_all[:, 8 * D:9 * D] += res.results[i]["G2"].astype(np.float64)
            G_all[:, 9 * D:] += res.results[i]["G3"].astype(np.float64)
        Gk = np.stack([G_all[:, k * D:(k + 1) * D] for k in range(K)])
        Gram = Gk.sum(axis=0)
    return Gk, Gram


def _epilogue(Gk, Gram, Pi, A, U):
    d_f = float(D)
    n_f = float(N)

    Mat = np.eye(D, dtype=np.float64) + (d_f / (n_f * EPS)) * Gram
    _, logdet = np.linalg.slogdet(Mat)
    loss_R = 0.5 * logdet

    trPi = Pi.astype(np.float64).sum(axis=0)            # [K]
    scalar = d_f / (trPi * EPS)
    Ar = np.maximum(A.astype(np.float64), 0.0)          # [K, D]
    logdets = np.log1p(scalar[:, None] * Ar).sum(axis=1)
    loss_Rc = 0.5 * np.sum(logdets * trPi) / n_f

    norms = np.maximum(np.linalg.norm(U, axis=0, keepdims=True), 1e-12)
    Un = (U / norms).astype(np.float64)
    M = np.einsum("dp,kp,ep->kde", Un, Ar, Un)
    loss_reg = 0.5 * np.sum((Gk - M) ** 2)

    loss_obj = loss_R - loss_Rc - MU * loss_reg
    return (
        np.float32(-loss_obj),
        np.float32(loss_R),
        np.float32(loss_Rc),
        np.float32(loss_reg),
    )


def kernel(Z, Pi, A, U):
    Z = np.asarray(Z, dtype=np.float32)
    Pi = np.asarray(Pi, dtype=np.float32)
    A = np.asarray(A, dtype=np.float32)
    U = np.asarray(U, dtype=np.float32)

    key, in_maps = _plan(Z, Pi)
    res = _run_device(key, in_maps)
    Gk, Gram = _gather(key, res)
    return _epilogue(Gk, Gram, Pi, A, U)


# revision 24
# speedup vs baseline: 1.0377x; 1.0377x over previous
"""MCR2 variational loss on 8 Trainium2 NeuronCores.

Math (reference):
  loss_R   = 0.5 * logdet(I + d/(n*eps) * Z.T @ Z)
  loss_Rc  = 0.5 * sum_k(trPi_k * sum_p log1p(d/(trPi_k*eps) * relu(A)_kp)) / n
  loss_reg = 0.5 * sum_k || G_k - Un diag(relu(A)_k) Un.T ||_F^2,
             G_k = Z.T diag(Pi[:,k]) Z
  out = (-(loss_R - loss_Rc - mu*loss_reg), loss_R, loss_Rc, loss_reg)

Fast path (Pi one-hot, the setup_inputs case): rows partition into 10
classes, so each masked Gram is a plain Gram over that class's rows and
the full Gram (for loss_R) is the sum of the ten. The host sorts rows by
class, deals each class's rows evenly across the 8 cores, and zero-pads
each (class, core) block to whole 128-row chunks (zero rows contribute
nothing). Per core the device then streams ~100 bf16 chunks ([128, 128]
each, partition-major in DRAM so DMA reads are contiguous) and runs ONE
N=128 matmul per chunk, accumulating class k's chunks into a per-class
PSUM bank. As each class finishes, its [128,128] Gram is copied to SBUF
(cast to bf16) and DMA'd out on the scalar HWDGE ring while later
classes still compute. Host sums the 8 partial G stacks and runs the
tiny O(k*d^2) epilogue (slogdet, compress, reg) in float64.

General path (Pi not one-hot): weighted-copy kernel — DVE builds
Pi_k * Z per chunk, wide matmuls accumulate the 10 masked Grams and the
full Gram.
"""

import math
import sys

if "/opt/trn_rl_repo" not in sys.path:
    sys.path.insert(0, "/opt/trn_rl_repo")

import ml_dtypes
import numpy as np

import concourse.bacc as bacc
import concourse.mybir as mybir
import concourse.tile as tile
from concourse import bass_utils

# Problem constants (hardcoded per harness contract).
N, D, K = 100000, 128, 10
EPS, MU = 0.5, 1.0
N_CORES = 8

_NC_CACHE = {}

# Device input dtype for the sorted fast path. fp8e4m3 halves HBM input
# traffic; host-simulated quantization error on the final losses is
# ~1.5e-3, far inside the 2e-2 gate (bf16: ~1e-6).
IN_DT = "fp8"  # "fp8" | "bf16"


# ---------------------------------------------------------------------------
# Fast path: Pi is one-hot -> sort rows by class, plain per-class Grams.
# ---------------------------------------------------------------------------

N_WARM = 12  # PE warmup matmuls (HAM clock-gate) while the first DMA lands


def _group_sizes(c_tot):
    """Input DMA slab sizes: small first slabs so compute starts early,
    16-chunk middle slabs, small final slab so the last chunks'
    completion semaphore doesn't gate the pipeline drain. Slabs
    alternate between the two HWDGE rings in consumption order, so
    neither ring runs far ahead of what the PE needs next."""
    head = [6, 8, 12, 16]
    tail = [24, 14]
    if c_tot <= sum(head) + sum(tail):
        sizes = []
        rem = c_tot
        while rem > 0:
            s = min(8, rem)
            sizes.append(s)
            rem -= s
        return sizes
    rem = c_tot - sum(head) - sum(tail)
    mid = []
    while rem > 0:
        s = min(20, rem)
        mid.append(s)
        rem -= s
    return head + mid + tail


def _build_nc_sorted(layout):
    """layout: tuple of 10 ints — 128-row chunks per class per core."""
    f32 = mybir.dt.float32
    bf16 = mybir.dt.bfloat16

    c_tot = sum(layout)
    starts = [0] * K
    for k in range(1, K):
        starts[k] = starts[k - 1] + layout[k - 1]
    class_of = []
    for k in range(K):
        class_of += [k] * layout[k]

    in_dt = mybir.dt.float8e4 if IN_DT == "fp8" else bf16

    nc = bacc.Bacc("TRN2", target_bir_lowering=False, debug=False)
    # Partition-major: row p holds chunk-row p of every chunk, so a slab
    # DMA reads a contiguous [128, g*128] region per partition.
    Zin = nc.dram_tensor("Zin", [128, c_tot * D], in_dt, kind="ExternalInput")
    # Separate output tensors per PSUM bank so the output DMAs have no
    # WAW dependency on each other and stream in parallel.
    G0 = nc.dram_tensor("G0", [128, 4 * D], bf16, kind="ExternalOutput")
    G1 = nc.dram_tensor("G1", [128, 4 * D], bf16, kind="ExternalOutput")
    G2 = nc.dram_tensor("G2", [128, D], bf16, kind="ExternalOutput")
    G3 = nc.dram_tensor("G3", [128, D], bf16, kind="ExternalOutput")

    with tile.TileContext(nc) as tc:
        with (
            tc.tile_pool(name="zb", bufs=1) as zpool,
            tc.tile_pool(name="res", bufs=1) as opool,
            tc.tile_pool(name="warm", bufs=1) as warmpool,
            tc.tile_pool(name="ps", bufs=5, space="PSUM") as pspool,
        ):
            # Whole shard lives in SBUF (25KB/partition); slab DMAs write
            # disjoint column ranges so matmuls on early slabs overlap
            # later slabs' transfers.
            zin = zpool.tile([128, c_tot * D], in_dt, name="zin")
            out = opool.tile([128, K * D], bf16, name="out")

            # PE warmup: the HAM clock gate only lifts to 2.4 GHz after
            # ~3.4µs of gap-free PE activity, and only re-throttles after
            # ~3.4µs of continuous idle. So: a dense warmup burst while
            # the first DMAs land, then short gap-filler matmuls between
            # the first slabs' chunks to bridge early data hiccups until
            # the window fires. After that the whole kernel runs warm.
            wsrc = warmpool.tile([128, 256], bf16, name="wsrc")
            wps = pspool.tile([128, 256], f32, name="wps", tag="ps")
            nc.gpsimd.memset(wsrc[:], 0.0)
            for _ in range(N_WARM):
                nc.tensor.matmul(wps[:], wsrc[:, 0:128], wsrc[:], start=True,
                                 stop=True, skip_group_check=True)

            def filler(n):
                for _ in range(n):
                    nc.tensor.matmul(wps[:, 0:128], wsrc[:, 0:128],
                                     wsrc[:, 0:128], start=True, stop=True,
                                     skip_group_check=True)

            # Input slabs alternate between the two HWDGE rings (SP via
            # nc.sync, ACT via nc.scalar) so descriptor generation and
            # packet streams run on both in parallel.
            g0 = 0
            for gi, sz in enumerate(_group_sizes(c_tot)):
                eng = nc.sync if gi % 2 == 0 else nc.scalar
                eng.dma_start(zin[:, g0 * D:(g0 + sz) * D],
                              Zin[:, g0 * D:(g0 + sz) * D])
                g0 += sz

            # Classes pack 4-per-PSUM-bank (classes run sequentially, so
            # a later class's group-start bank-clear only touches
            # finished classes' has_written bits, never live data). One
            # cast + one output DMA per bank: no PSUM slot cycling (no
            # PE drains), and the PE is always writing a different bank
            # than the one the DVE reads. Early banks' outputs are
            # hidden under compute on the sync ring; the last bank rides
            # the scalar ring on the tail.
            psA = pspool.tile([128, 4 * D], f32, name="psA", tag="ps")
            psB = pspool.tile([128, 4 * D], f32, name="psB", tag="ps")
            psC = pspool.tile([128, D], f32, name="psC", tag="ps")
            psD = pspool.tile([128, D], f32, name="psD", tag="ps")
            banks = {k: (psA, k) for k in range(4)}
            banks.update({k: (psB, k - 4) for k in range(4, 8)})
            banks[8] = (psC, 0)
            banks[9] = (psD, 0)
            bank_last = {3: (psA, 4, G0, nc.sync),
                         7: (psB, 4, G1, nc.sync),
                         8: (psC, 1, G2, nc.scalar),
                         9: (psD, 1, G3, nc.scalar)}

            # Gap fillers after the first slabs' chunks (slab boundaries
            # are where early data hiccups stall the PE).
            slab_fill = {0: 3, 1: 2, 2: 2, 3: 1}
            slab_bounds = []
            g0 = 0
            for sz in _group_sizes(c_tot):
                slab_bounds.append(g0 + sz)
                g0 += sz

            for idx in range(c_tot):
                k = class_of[idx]
                j = idx - starts[k]
                pst, off = banks[k]
                zc = zin[:, idx * D:(idx + 1) * D]
                nc.tensor.matmul(pst[:, off * D:(off + 1) * D], zc, zc,
                                 start=(j == 0), stop=(j == layout[k] - 1),
                                 skip_group_check=True)
                if j == layout[k] - 1 and k in bank_last:
                    pst, width, Gt, eng = bank_last[k]
                    osl = out[:, (k + 1 - width) * D:(k + 1) * D]
                    nc.vector.tensor_copy(osl, pst[:, 0:width * D])
                    eng.dma_start(Gt[:], osl)
                if idx + 1 in slab_bounds:
                    si = slab_bounds.index(idx + 1)
                    if si in slab_fill:
                        filler(slab_fill[si])

    nc.compile()
    return nc


def _layout_from_counts(counts):
    return tuple(
        max(1, math.ceil(math.ceil(int(c) / N_CORES) / 128)) for c in counts
    )


def _make_in_maps_sorted(Z, labels, counts, layout):
    c_tot = sum(layout)
    chunk_off = np.concatenate([[0], np.cumsum(layout)]).astype(np.int64)
    order = np.argsort(labels, kind="stable")
    np_dt = ml_dtypes.float8_e4m3 if IN_DT == "fp8" else ml_dtypes.bfloat16
    Zb = Z.astype(np_dt)

    per_core = np.zeros((N_CORES, c_tot * 128, D), np_dt)
    cum = 0
    for k in range(K):
        rows_k = order[cum:cum + counts[k]]
        cum += counts[k]
        q, r = divmod(int(counts[k]), N_CORES)
        s = 0
        base = int(chunk_off[k]) * 128
        for c in range(N_CORES):
            n_c = q + (1 if c < r else 0)
            if n_c:
                per_core[c, base:base + n_c] = Zb[rows_k[s:s + n_c]]
            s += n_c
    Zin = np.ascontiguousarray(
        per_core.reshape(N_CORES, c_tot, 128, D)
        .transpose(0, 2, 1, 3)
        .reshape(N_CORES, 128, c_tot * D)
    )
    return [{"Zin": Zin[c]} for c in range(N_CORES)]


# ---------------------------------------------------------------------------
# General path (Pi not one-hot): weighted-copy kernel from the baseline.
# ---------------------------------------------------------------------------

CHUNKS = 98                    # 128-row chunks per core
SHARD = CHUNKS * 128           # 12544 rows per core
NPAD = SHARD * N_CORES         # 100352 (zero-padded; zero rows contribute 0)
GROUP = 7                      # chunks per staged DMA group
NCLS = K + 1                   # 10 masked Grams + 1 full Gram


def _build_nc_general():
    f32 = mybir.dt.float32
    bf16 = mybir.dt.bfloat16

    nc = bacc.Bacc("TRN2", target_bir_lowering=False, debug=False)
    # Per-row payload: [Pi7*Z | Pi8*Z | Pi9*Z | Z_bf16] — classes 7..9
    # weighted on host. One DMA per group feeds everything; a single
    # N=512 matmul over the whole row computes G7, G8, G9 and the Gram.
    ZW = nc.dram_tensor("ZW", [SHARD, 4 * D], bf16, kind="ExternalInput")
    KD = K - 3  # classes 0..6 weighted on DVE; 7..9 host-weighted
    Pb = nc.dram_tensor("Pb", [128, CHUNKS, KD, 2], bf16, kind="ExternalInput")
    G = nc.dram_tensor("G", [D, NCLS * D], f32, kind="ExternalOutput")

    with tile.TileContext(nc) as tc:
        with (
            tc.tile_pool(name="zbf", bufs=6) as zbpool,
            tc.tile_pool(name="wgt", bufs=4) as wpool,
            tc.tile_pool(name="pi", bufs=1) as pipool,
            tc.tile_pool(name="res", bufs=1) as opool,
            tc.tile_pool(name="warm", bufs=1) as warmpool,
            tc.tile_pool(name="ps", bufs=1, space="PSUM") as pspool,
        ):
            psA = pspool.tile([128, 512], f32, name="psA")
            psB = pspool.tile([128, 384], f32, name="psB")
            psC = pspool.tile([128, 512], f32, name="psC")

            wsrc = warmpool.tile([128, 256], bf16, name="wsrc")
            wps = pspool.tile([128, 256], f32, name="wps")
            nc.gpsimd.memset(wsrc[:], 0.0)
            for _ in range(22):
                nc.tensor.matmul(wps[:], wsrc[:, 0:128], wsrc[:], start=True,
                                 stop=True, skip_group_check=True)

            Zr = ZW.rearrange("(c p) d -> p c d", p=128)

            pib = pipool.tile([128, CHUNKS, KD, 2], bf16, name="pib")
            nc.gpsimd.dma_start(pib[:, 0:1], Pb[:, 0:1])
            nc.gpsimd.dma_start(pib[:, 1:8], Pb[:, 1:8])
            nc.gpsimd.dma_start(pib[:, 8:29], Pb[:, 8:29])
            nc.gpsimd.dma_start(pib[:, 29:CHUNKS], Pb[:, 29:CHUNKS])

            sizes = [1] + [GROUP] * 12 + [5, 4, 3, 1]
            assert sum(sizes) == CHUNKS

            start_c = 0
            for gi, sz in enumerate(sizes):
                s0 = start_c
                start_c += sz
                zw = zbpool.tile([128, sz, 4 * D], bf16, name="zw", tag="zw")
                nc.sync.dma_start(zw[:], Zr[:, s0:s0 + sz, :])
                zb = zw[:, :, 3 * D:4 * D]

                wg = wpool.tile([128, sz, KD * D], bf16, name="wg", tag="wg")
                z_bc = zb.unsqueeze(2).broadcast_to([128, sz, KD, D])
                pi_bc = (
                    pib[:, s0:s0 + sz, :, :]
                    .unsqueeze(3)
                    .broadcast_to([128, sz, KD, 64, 2])
                )
                w5 = wg[:, :, 0:KD * D].rearrange(
                    "p c (k r t) -> p c k r t", k=KD, t=2
                )
                z5 = z_bc.rearrange("p c k (r t) -> p c k r t", t=2)
                nc.vector.tensor_mul(w5, z5, pi_bc)

                for c in range(sz):
                    idx = s0 + c
                    first = idx == 0
                    last = idx == CHUNKS - 1
                    zc = zw[:, c, 3 * D:4 * D]
                    w = wg[:, c, :]
                    nc.tensor.matmul(psA[:], zc, w[:, 0:512], start=first, stop=last)
                    nc.tensor.matmul(psB[:], zc, w[:, 512:896], start=first, stop=last)
                    nc.tensor.matmul(psC[:], zc, zw[:, c, :], start=first, stop=last)

                if gi <= 3:
                    for _ in range((10, 4, 3, 3)[gi]):
                        nc.tensor.matmul(wps[:], wsrc[:, 0:128], wsrc[:],
                                         start=True, stop=True,
                                         skip_group_check=True)

            out = opool.tile([128, NCLS * D], f32, name="out")
            nc.vector.tensor_copy(out[:, 0:512], psA[:])
            nc.scalar.copy(out[:, 512:896], psB[:])
            nc.sync.dma_start(G[:, 0:896], out[:, 0:896])
            nc.vector.tensor_copy(out[:, 896:1408], psC[:])
            nc.sync.dma_start(G[:, 896:1408], out[:, 896:1408])

    nc.compile()
    return nc


def _make_in_maps_general(Z, Pi):
    ZWpad = np.zeros((NPAD, 4 * D), ml_dtypes.bfloat16)
    for j in range(3):
        ZWpad[:N, j * D:(j + 1) * D] = (
            Pi[:, K - 3 + j:K - 2 + j] * Z
        ).astype(ml_dtypes.bfloat16)
    ZWpad[:N, 3 * D:4 * D] = Z.astype(ml_dtypes.bfloat16)
    Pipad = np.zeros((NPAD, K), np.float32)
    Pipad[:N] = Pi
    in_maps = []
    for i in range(N_CORES):
        zw = np.ascontiguousarray(ZWpad[i * SHARD:(i + 1) * SHARD])
        pt = (
            Pipad[i * SHARD:(i + 1) * SHARD, 0:K - 3]
            .reshape(CHUNKS, 128, K - 3)
            .transpose(1, 0, 2)
            .astype(ml_dtypes.bfloat16)
        )
        pb = np.ascontiguousarray(np.repeat(pt[..., None], 2, axis=-1))
        in_maps.append({"ZW": zw, "Pb": pb})
    return in_maps


# ---------------------------------------------------------------------------
# Shared driver / epilogue
# ---------------------------------------------------------------------------


def _get_nc(key):
    if key not in _NC_CACHE:
        if key == "general":
            _NC_CACHE[key] = _build_nc_general()
        else:
            _NC_CACHE[key] = _build_nc_sorted(key[1])
    return _NC_CACHE[key]


def _is_one_hot(Pi):
    if not np.all((Pi == 0.0) | (Pi == 1.0)):
        return False
    return bool(np.all(Pi.sum(axis=1) == 1.0))


def _plan(Z, Pi):
    """Returns (key, in_maps) for the right device path."""
    if _is_one_hot(Pi):
        labels = np.argmax(Pi, axis=1)
        counts = np.bincount(labels, minlength=K)
        layout = _layout_from_counts(counts)
        key = ("sorted", layout)
        in_maps = _make_in_maps_sorted(Z, labels, counts, layout)
    else:
        key = "general"
        in_maps = _make_in_maps_general(Z, Pi)
    return key, in_maps


def _run_device(key, in_maps, trace=False, tmpdir=None):
    nc = _get_nc(key)
    return bass_utils.run_bass_kernel_spmd(
        nc, in_maps, core_ids=list(range(N_CORES)), trace=trace, tmpdir=tmpdir
    )


def _gather(key, res):
    """Sum per-core partial Grams -> (Gk [K,D,D], Gram [D,D]) in float64."""
    if key == "general":
        G_all = np.zeros((D, NCLS * D), np.float64)
        for i in range(N_CORES):
            G_all += res.results[i]["G"].astype(np.float64)
        Gk = np.stack([G_all[:, k * D:(k + 1) * D] for k in range(K)])
        Gram = G_all[:, K * D:(K + 1) * D]
    else:
        G_all = np.zeros((D, K * D), np.float64)
        for i in range(N_CORES):
            G_all[:, 0:4 * D] += res.results[i]["G0"].astype(np.float64)
            G_all[:, 4 * D:8 * D] += res.results[i]["G1"].astype(np.float64)
            G_all[:, 8 * D:9 * D] += res.results[i]["G2"].astype(np.float64)
            G_all[:, 9 * D:] += res.results[i]["G3"].astype(np.float64)
        Gk = np.stack([G_all[:, k * D:(k + 1) * D] for k in range(K)])
        Gram = Gk.sum(axis=0)
    return Gk, Gram


def _epilogue(Gk, Gram, Pi, A, U):
    d_f = float(D)
    n_f = float(N)

    Mat = np.eye(D, dtype=np.float64) + (d_f / (n_f * EPS)) * Gram
    _, logdet = np.linalg.slogdet(Mat)
    loss_R = 0.5 * logdet

    trPi = Pi.astype(np.float64).sum(axis=0)            # [K]
    scalar = d_f / (trPi * EPS)
    Ar = np.maximum(A.astype(np.float64), 0.0)          # [K, D]
    logdets = np.log1p(scalar[:, None] * Ar).sum(axis=1)
    loss_Rc = 0.5 * np.sum(logdets * trPi) / n_f

    norms = np.maximum(np.linalg.norm(U, axis=0, keepdims=True), 1e-12)
    Un = (U / norms).astype(np.float64)
    M = np.einsum("dp,kp,ep->kde", Un, Ar, Un)
    loss_reg = 0.5 * np.sum((Gk - M) ** 2)

    loss_obj = loss_R - loss_Rc - MU * loss_reg
    return (
        np.float32(-loss_obj),
        np.float32(loss_R),
        np.float32(loss_Rc),
        np.float32(loss_reg),
    )


def kernel(Z, Pi, A, U):
    Z = np.asarray(Z, dtype=np.float32)
    Pi = np.asarray(Pi, dtype=np.float32)
    A = np.asarray(A, dtype=np.float32)
    U = np.asarray(U, dtype=np.float32)

    key, in_maps = _plan(Z, Pi)
    res = _run_device(key, in_maps)
    Gk, Gram = _gather(key, res)
    return _epilogue(Gk, Gram, Pi, A, U)
